# revision 8
# baseline (speedup 1.0000x reference)
"""Trainium2 Bass kernel for nn_DecoderLayer_88424786690751 (MoE decoder layer).

8-core SPMD strategy:
  - ln1/ln2 folded into consuming weights host-side; RoPE folded into augmented
    projection weights: scores accumulate two K=128 matmuls
    (kaug1.T@qaug + kaug2.T@qaug == rope(k) @ rope(q)^T exactly).
  - Attention: 24 (batch, head) jobs -> 3 heads/core, cores 0-3 batch 0,
    cores 4-7 batch 1. Transposed activations [d, t]. Causal block skipping;
    diagonal blocks masked multiplicatively post-exp (host-passed 0/1 tiles).
    Softmax denominator from a ones column stacked into v (vaug).
    Batch selection is pure data: group AllGather [[0-3],[4-7]] feeds each
    core its batch's tokens at identical offsets; wo outputs are written to
    both batch column-halves scaled by a per-core 0/1 alpha mask so a single
    global ReduceScatter(add) sums partials correctly.
  - Router replicated over all tokens (tiny); per-core expert row selected by
    a one-hot matmul. MoE dense expert-parallel: core e computes expert e on
    all tokens, scaled by its routing-weight column (0 for unrouted tokens,
    so this matches top-2 routing exactly).
  - Two global ReduceScatters [768,4096]->[96,4096] (attention, MoE); host
    concatenates the 8 row-shards and transposes back.
All matmuls fp32r (full PE rate at N>=256, ~2e-4 relative precision).
"""
import sys
if '/opt/trn_rl_repo' not in sys.path:
    sys.path.insert(0, '/opt/trn_rl_repo')

from contextlib import ExitStack

import numpy as np

import concourse.bass as bass
import concourse.mybir as mybir
import concourse.tile as tile
from concourse import bacc
from concourse.bass_utils import run_bass_kernel_spmd
from concourse.masks import make_identity

F32 = mybir.dt.float32
F32R = mybir.dt.float32r
AF = mybir.ActivationFunctionType
ALU = mybir.AluOpType
ts = bass.ts

H, NH, HD, II, E = 768, 12, 64, 2048, 8
B, S = 2, 2048
T = B * S          # 4096
NC = 8
EPS = 1e-6
Z_LOSS = 0.001
ROPE_BASE = 10000.0
KT = H // 128       # 6   H k-tiles
ISL = II // 128     # 16  I-slices
QCH = S // 512      # 4   per-batch 512-token chunks
SKT = S // 128      # 16  per-batch 128-token key tiles
RSH = H // NC       # 96  H-row shard
TSH = T // NC       # 512 token shard

_CACHE = {}


def _emit(nc):
    din = lambda n, s: nc.declare_dram_parameter(n, list(s), F32, isOutput=False)
    h_shard = din("h_shard", (TSH, H))
    hT_rows = din("hT_rows", (RSH, T))
    pq_in = din("pq", (3, H, 128))
    pk1_in = din("pk1", (3, H, 128))
    pk2_in = din("pk2", (3, H, 128))
    pv_ab = din("pv_ab", (H, 128))
    pv_c = din("pv_c", (H, HD))
    wo3 = din("wo3", (3, HD, H))
    cs1 = din("cs1", (128, S))
    cs2 = din("cs2", (128, S))
    extm = din("extm", (128, 896))
    gatew = din("gatew", (H, E))
    esel = din("esel", (E, 1))
    amask = din("amask", (128, 2))
    wg = din("wg", (H, II))
    wu = din("wu", (H, II))
    wd = din("wd", (II, H))
    out_shard = nc.declare_dram_parameter("out_shard", [RSH, T], F32, isOutput=True)
    aux_out = nc.declare_dram_parameter("aux_out", [1, 1], F32, isOutput=True)

    r_ = lambda ap: ap.bitcast(F32R)

    with tile.TileContext(nc) as tc, ExitStack() as top:
        pers = top.enter_context(tc.tile_pool(name="pers", bufs=1))
        dram = top.enter_context(tc.tile_pool(name="dram", bufs=1, space="DRAM"))

        ident = pers.tile([128, 128], F32)
        make_identity(nc, ident[:])
        ones_f = pers.tile([128, 1], F32)
        nc.vector.memset(ones_f[:], 1.0)
        onesr_col = pers.tile([128, 1], F32R)       # ones lhsT [K<=128, 1]
        nc.vector.tensor_copy(onesr_col[:], ones_f[:])
        ones_fr = pers.tile([1, 128], F32)
        nc.vector.memset(ones_fr[:], 1.0)
        onesr_row = pers.tile([1, 128], F32R)       # ones lhsT [1, M<=128]
        nc.vector.tensor_copy(onesr_row[:], ones_fr[:])
        eps_sb = pers.tile([128, 1], F32)
        nc.vector.memset(eps_sb[:], EPS)
        am_sb = pers.tile([128, 2], F32)
        nc.sync.dma_start(am_sb[:], amask[:])

        combrow_d = dram.tile([1, T], F32)          # my expert's comb weights
        x1T_d = dram.tile([RSH, T], F32)            # hidden+attn rows

        # ============ Phase 0: ln1 rmsnorm + transpose + group AllGather ====
        xnT_shard_d = dram.tile([H, TSH], F32)
        xnT_ag = dram.tile([4, H, TSH], F32)
        with ExitStack() as ph:
            sb = ph.enter_context(tc.tile_pool(name="p0sb", bufs=2))
            ps = ph.enter_context(tc.tile_pool(name="p0ps", bufs=2, space="PSUM"))
            for j in range(TSH // 128):
                x_t = sb.tile([128, H], F32, tag="x")
                nc.sync.dma_start(x_t[:], h_shard[ts(j, 128), :])
                sq_t = sb.tile([128, H], F32, tag="sq")
                ss_t = sb.tile([128, 1], F32, tag="ss")
                nc.scalar.activation(sq_t[:], x_t[:], AF.Square, accum_out=ss_t[:])
                std_t = sb.tile([128, 1], F32, tag="sd")
                nc.scalar.activation(std_t[:], ss_t[:], AF.Sqrt,
                                     bias=eps_sb[:], scale=1.0 / H)
                rcp_t = sb.tile([128, 1], F32, tag="rc")
                nc.vector.reciprocal(rcp_t[:], std_t[:])
                xn_t = sb.tile([128, H], F32, tag="xn")
                nc.vector.tensor_scalar_mul(xn_t[:], x_t[:], rcp_t[:])
                for kk in range(KT):
                    tr_ps = ps.tile([128, 128], F32, tag="tr")
                    nc.tensor.transpose(tr_ps[:], xn_t[:, ts(kk, 128)], ident[:])
                    tr_sb = sb.tile([128, 128], F32, tag="tb")
                    nc.vector.tensor_copy(tr_sb[:], tr_ps[:])
                    nc.sync.dma_start(
                        xnT_shard_d[ts(kk, 128), ts(j, 128)], tr_sb[:])
        nc.gpsimd.collective_compute(
            "AllGather", ALU.bypass,
            replica_groups=[[0, 1, 2, 3], [4, 5, 6, 7]],
            ins=[xnT_shard_d[:].opt()], outs=[xnT_ag[:].opt()])

        # ============ Phase 1: attention ====================================
        attn_rs_in = dram.tile([H, T], F32)
        attn_rs_out = dram.tile([RSH, T], F32)
        with ExitStack() as ph:
            sbw = ph.enter_context(tc.tile_pool(name="p1w", bufs=1))
            sb = ph.enter_context(tc.tile_pool(name="p1sb", bufs=2))
            ps = ph.enter_context(tc.tile_pool(name="p1ps", bufs=2, space="PSUM"))
            psav = ph.enter_context(tc.tile_pool(name="p1av", bufs=2, space="PSUM"))
            pswo = ph.enter_context(tc.tile_pool(name="p1wo", bufs=2, space="PSUM"))
            psbc = ph.enter_context(tc.tile_pool(name="p1bc", bufs=1, space="PSUM"))

            xq_sb = sbw.tile([128, KT, QCH, 512], F32R)   # my batch's xnT
            for kk in range(KT):
                for q in range(QCH):
                    nc.sync.dma_start(xq_sb[:, kk, q, :],
                                      r_(xnT_ag[q, ts(kk, 128), :]))
            pvab_sb = sbw.tile([128, KT, 128], F32R)
            pvc_sb = sbw.tile([128, KT, HD], F32R)
            for kk in range(KT):
                nc.sync.dma_start(pvab_sb[:, kk, :], r_(pv_ab[ts(kk, 128), :]))
                nc.sync.dma_start(pvc_sb[:, kk, :], r_(pv_c[ts(kk, 128), :]))
            wo_sb = sbw.tile([HD, 3, KT, 128], F32R)
            for i in range(3):
                for kk in range(KT):
                    nc.sync.dma_start(wo_sb[:, i, kk, :], r_(wo3[i, :, ts(kk, 128)]))
            cs1_sb = sbw.tile([128, S], F32R)
            cs2_sb = sbw.tile([128, S], F32R)
            nc.sync.dma_start(cs1_sb[:], r_(cs1[:]))
            nc.sync.dma_start(cs2_sb[:], r_(cs2[:]))
            dm_sb = sbw.tile([128, 896], F32R)
            nc.sync.dma_start(dm_sb[:], r_(extm[:]))

            # --- v for all 3 heads, then vaug = [v | 1] transposed back ---
            vaug_sb = sbw.tile([128, SKT, 3, 65], F32R)
            for q in range(QCH):
                vab_ps = ps.tile([128, 512], F32, tag="sc")
                for kk in range(KT):
                    nc.tensor.matmul(vab_ps[:], pvab_sb[:, kk, :],
                                     xq_sb[:, kk, q, :],
                                     start=(kk == 0), stop=(kk == KT - 1))
                vabq = sb.tile([128, 512], F32, tag="vab")
                nc.vector.tensor_copy(vabq[:], vab_ps[:])
                vc_ps = ps.tile([HD, 512], F32, tag="sc")
                for kk in range(KT):
                    nc.tensor.matmul(vc_ps[:], pvc_sb[:, kk, :],
                                     xq_sb[:, kk, q, :],
                                     start=(kk == 0), stop=(kk == KT - 1))
                vcq = sb.tile([HD, 512], F32, tag="vc")
                nc.vector.tensor_copy(vcq[:], vc_ps[:])
                for sub in range(4):
                    kt = 4 * q + sub
                    trab_ps = psav.tile([128, 128], F32, tag="av")
                    nc.tensor.transpose(trab_ps[:], vabq[:, ts(sub, 128)],
                                        ident[:])
                    nc.vector.tensor_copy(vaug_sb[:, kt, 0, 0:64],
                                          trab_ps[:, 0:64])
                    nc.vector.tensor_copy(vaug_sb[:, kt, 1, 0:64],
                                          trab_ps[:, 64:128])
                    trc_ps = psav.tile([128, 64], F32, tag="av")
                    nc.tensor.transpose(trc_ps[:], vcq[:, ts(sub, 128)],
                                        ident[0:64, 0:64])
                    nc.vector.tensor_copy(vaug_sb[:, kt, 2, 0:64], trc_ps[:])
                    for i in range(3):
                        nc.vector.tensor_copy(vaug_sb[:, kt, i, 64:65],
                                              ones_f[:])

            # --- per pair: augmented projections, then scores/av/normalize ---
            outT_st = sbw.tile([HD, 3, QCH, 512], F32R)   # normalized heads
            for i in range(3):
                pqi_sb = sbw.tile([128, KT, 128], F32R, tag="pqi")
                pk1i_sb = sbw.tile([128, KT, 128], F32R, tag="pk1i")
                pk2i_sb = sbw.tile([128, KT, 128], F32R, tag="pk2i")
                for kk in range(KT):
                    nc.sync.dma_start(pqi_sb[:, kk, :], r_(pq_in[i, ts(kk, 128), :]))
                    nc.sync.dma_start(pk1i_sb[:, kk, :], r_(pk1_in[i, ts(kk, 128), :]))
                    nc.sync.dma_start(pk2i_sb[:, kk, :], r_(pk2_in[i, ts(kk, 128), :]))
                qaug_sb = sbw.tile([128, S], F32R, tag="qaug")
                kaug1_sb = sbw.tile([128, S], F32R, tag="kaug1")
                kaug2_sb = sbw.tile([128, S], F32R, tag="kaug2")
                for q in range(QCH):
                    for (w_sb, cs_sb, dst) in (
                        (pqi_sb, cs1_sb, qaug_sb),
                        (pk1i_sb, cs1_sb, kaug1_sb),
                        (pk2i_sb, cs2_sb, kaug2_sb),
                    ):
                        pr_ps = ps.tile([128, 512], F32, tag="sc")
                        for kk in range(KT):
                            nc.tensor.matmul(pr_ps[:], w_sb[:, kk, :],
                                             xq_sb[:, kk, q, :],
                                             start=(kk == 0), stop=(kk == KT - 1))
                        nc.vector.tensor_tensor(
                            dst[:, ts(q, 512)], pr_ps[:],
                            cs_sb[:, ts(q, 512)], op=ALU.mult)
                for q in range(QCH):
                    nkt = 4 * q + 4
                    av_ps = psav.tile([65, 512], F32, tag="av")
                    for kt in range(nkt):
                        sc_ps = ps.tile([128, 512], F32, tag="sc")
                        nc.tensor.matmul(sc_ps[:], kaug1_sb[:, ts(kt, 128)],
                                         qaug_sb[:, ts(q, 512)],
                                         start=True, stop=False)
                        nc.tensor.matmul(sc_ps[:], kaug2_sb[:, ts(kt, 128)],
                                         qaug_sb[:, ts(q, 512)],
                                         start=False, stop=True)
                        pr_sb = sb.tile([128, 512], F32R, tag="pr")
                        nc.scalar.activation(pr_sb[:], sc_ps[:], AF.Exp)
                        m = kt - 4 * q
                        if m >= 0:
                            nc.vector.tensor_tensor(
                                pr_sb[:], pr_sb[:],
                                dm_sb[:, bass.ds((3 - m) * 128, 512)],
                                op=ALU.mult)
                        nc.tensor.matmul(av_ps[:], vaug_sb[:, kt, i, :], pr_sb[:],
                                         start=(kt == 0), stop=(kt == nkt - 1))
                    o_sb = sb.tile([65, 512], F32, tag="o")
                    nc.vector.tensor_copy(o_sb[:], av_ps[:])
                    den_sb = sb.tile([1, 512], F32, tag="dn")
                    nc.sync.dma_start(den_sb[:], o_sb[64:65, :])
                    rcp_sb = sb.tile([1, 512], F32, tag="rcd")
                    nc.vector.reciprocal(rcp_sb[:], den_sb[:])
                    rcpr_sb = sb.tile([1, 512], F32R, tag="rcr")
                    nc.vector.tensor_copy(rcpr_sb[:], rcp_sb[:])
                    bc_ps = psbc.tile([64, 512], F32, tag="bc")
                    nc.tensor.matmul(bc_ps[:], onesr_row[:, 0:64], rcpr_sb[:],
                                     start=True, stop=True)
                    nc.vector.tensor_tensor(outT_st[:, i, q, :], o_sb[0:64, :],
                                            bc_ps[:], op=ALU.mult)
            # --- wo projection, alpha-masked double write ---
            for q in range(QCH):
                for kk in range(KT):
                    wo_ps = pswo.tile([128, 512], F32, tag="wo")
                    for i in range(3):
                        nc.tensor.matmul(wo_ps[:], wo_sb[:, i, kk, :],
                                         outT_st[:, i, q, :],
                                         start=(i == 0), stop=(i == 2))
                    for bb in range(2):
                        aw_sb = sb.tile([128, 512], F32, tag="aw")
                        nc.vector.tensor_scalar_mul(aw_sb[:], wo_ps[:],
                                                    am_sb[:, bb:bb + 1])
                        nc.sync.dma_start(
                            attn_rs_in[ts(kk, 128),
                                       bass.ds(2048 * bb + 512 * q, 512)],
                            aw_sb[:])
        nc.gpsimd.collective_compute(
            "ReduceScatter", ALU.add, replica_groups=[list(range(NC))],
            ins=[attn_rs_in[:].opt()], outs=[attn_rs_out[:].opt()])

        # ============ Phase 2: residual + ln2 on [96, 4096] row shard =======
        x2n_shard_d = dram.tile([RSH, T], F32)
        x2nT_ag = dram.tile([NC, RSH, T], F32, addr_space="Shared")
        ss_d = dram.tile([1, T], F32)
        ss_ag = dram.tile([NC, 1, T], F32, addr_space="Shared")
        with ExitStack() as ph:
            sb = ph.enter_context(tc.tile_pool(name="p2sb", bufs=2))
            sbp = ph.enter_context(tc.tile_pool(name="p2sbp", bufs=1))
            ps = ph.enter_context(tc.tile_pool(name="p2ps", bufs=2, space="PSUM"))
            ss_sb = sbp.tile([1, T], F32)
            for c in range(T // 512):
                ar_t = sb.tile([RSH, 512], F32, tag="ar")
                nc.sync.dma_start(ar_t[:], attn_rs_out[:, ts(c, 512)])
                ht_t = sb.tile([RSH, 512], F32, tag="ht")
                nc.sync.dma_start(ht_t[:], hT_rows[:, ts(c, 512)])
                x1c = sb.tile([RSH, 512], F32, tag="x1")
                nc.vector.tensor_tensor(x1c[:], ar_t[:], ht_t[:], op=ALU.add)
                nc.sync.dma_start(x1T_d[:, ts(c, 512)], x1c[:])
                sq_t = sb.tile([RSH, 512], F32R, tag="sq")
                nc.scalar.activation(sq_t[:], x1c[:], AF.Square)
                ssp_ps = ps.tile([1, 512], F32, tag="ss")
                nc.tensor.matmul(ssp_ps[:], onesr_col[0:RSH, :], sq_t[:],
                                 start=True, stop=True)
                nc.vector.tensor_copy(ss_sb[:, ts(c, 512)], ssp_ps[:])
            nc.sync.dma_start(ss_d[:], ss_sb[:])
            nc.gpsimd.collective_compute(
                "AllGather", ALU.bypass, replica_groups=[list(range(NC))],
                ins=[ss_d[:].opt()], outs=[ss_ag[:].opt()])
            ssall_sb = sbp.tile([NC, T], F32R)
            nc.sync.dma_start(ssall_sb[:],
                              r_(ss_ag[:].rearrange("r a t -> (r a) t")))
            rs2_sb = sbp.tile([1, T], F32)
            rs2r_sb = sbp.tile([1, T], F32R)
            for c in range(T // 512):
                var_ps = ps.tile([1, 512], F32, tag="ss")
                nc.tensor.matmul(var_ps[:], onesr_col[0:NC, :],
                                 ssall_sb[:, ts(c, 512)], start=True, stop=True)
                std_t = sb.tile([1, 512], F32, tag="sd")
                nc.scalar.activation(std_t[:], var_ps[:], AF.Sqrt,
                                     bias=eps_sb[0:1, :], scale=1.0 / H)
                nc.vector.reciprocal(rs2_sb[:, ts(c, 512)], std_t[:])
            nc.vector.tensor_copy(rs2r_sb[:], rs2_sb[:])
            for c in range(T // 512):
                bc_ps = ps.tile([RSH, 512], F32, tag="bc")
                nc.tensor.matmul(bc_ps[:], onesr_row[:, 0:RSH],
                                 rs2r_sb[:, ts(c, 512)], start=True, stop=True)
                x1b = sb.tile([RSH, 512], F32, tag="x1b")
                nc.sync.dma_start(x1b[:], x1T_d[:, ts(c, 512)])
                x2n_t = sb.tile([RSH, 512], F32, tag="x2")
                nc.vector.tensor_tensor(x2n_t[:], x1b[:], bc_ps[:],
                                        op=ALU.mult)
                nc.sync.dma_start(x2n_shard_d[:, ts(c, 512)], x2n_t[:])
        nc.gpsimd.collective_compute(
            "AllGather", ALU.bypass, replica_groups=[list(range(NC))],
            ins=[x2n_shard_d[:].opt()], outs=[x2nT_ag[:].opt()])
        x2nT_flat = x2nT_ag[:].rearrange("r p t -> (r p) t")  # [768, 4096]

        # ============ Phase 3: router (replicated over all tokens) ==========
        with ExitStack() as ph:
            sb = ph.enter_context(tc.tile_pool(name="p3sb", bufs=2))
            sbp = ph.enter_context(tc.tile_pool(name="p3sbp", bufs=1))
            ps = ph.enter_context(tc.tile_pool(name="p3ps", bufs=2, space="PSUM"))
            gw_sb = sbp.tile([128, KT, E], F32R)
            for kk in range(KT):
                nc.sync.dma_start(gw_sb[:, kk, :], r_(gatew[ts(kk, 128), :]))
            es_sb = sbp.tile([E, 1], F32R)
            nc.sync.dma_start(es_sb[:], r_(esel[:]))
            lsqrow = sbp.tile([1, T], F32)
            for tt in range(T // 128):
                xt_sb = sb.tile([128, KT, 128], F32R, tag="xt")
                for kk in range(KT):
                    nc.sync.dma_start(xt_sb[:, kk, :],
                                      r_(x2nT_flat[ts(kk, 128), ts(tt, 128)]))
                lg_ps = ps.tile([128, E], F32, tag="lg")
                for kk in range(KT):
                    nc.tensor.matmul(lg_ps[:], xt_sb[:, kk, :], gw_sb[:, kk, :],
                                     start=(kk == 0), stop=(kk == KT - 1))
                lg = sb.tile([128, E], F32, tag="l0")
                nc.vector.tensor_copy(lg[:], lg_ps[:])
                nm1 = sb.tile([128, 1], F32, tag="m1")
                nc.vector.tensor_reduce(nm1[:], lg[:], axis=mybir.AxisListType.X,
                                        op=ALU.max, negate=True)
                t1 = sb.tile([128, E], F32, tag="t1")
                nc.vector.tensor_scalar_add(t1[:], lg[:], nm1[:])
                is1 = sb.tile([128, E], F32, tag="i1")
                nc.vector.tensor_scalar(is1[:], t1[:], 0.0, None, op0=ALU.is_ge)
                big = sb.tile([128, E], F32, tag="bg")
                nc.vector.tensor_scalar_mul(big[:], is1[:], -1e9)
                t2 = sb.tile([128, E], F32, tag="t2")
                nc.vector.tensor_tensor(t2[:], t1[:], big[:], op=ALU.add)
                nm2 = sb.tile([128, 1], F32, tag="m2")
                nc.vector.tensor_reduce(nm2[:], t2[:], axis=mybir.AxisListType.X,
                                        op=ALU.max, negate=True)
                iz = sb.tile([128, E], F32, tag="iz")
                nc.vector.tensor_scalar_add(iz[:], t2[:], nm2[:])
                is2 = sb.tile([128, E], F32, tag="i2")
                nc.vector.tensor_scalar(is2[:], iz[:], 0.0, None, op0=ALU.is_ge)
                topm = sb.tile([128, E], F32, tag="tm")
                nc.vector.tensor_tensor(topm[:], is1[:], is2[:], op=ALU.add)
                em2 = sb.tile([128, 1], F32, tag="e2")
                nc.scalar.activation(em2[:], nm2[:], AF.Exp, scale=-1.0)
                den2 = sb.tile([128, 1], F32, tag="d2")
                nc.vector.tensor_scalar_add(den2[:], em2[:], 1.0)
                recd = sb.tile([128, 1], F32, tag="rd")
                nc.vector.reciprocal(recd[:], den2[:])
                unn = sb.tile([128, E], F32, tag="un")
                sume = sb.tile([128, 1], F32, tag="se")
                nc.scalar.activation(unn[:], t1[:], AF.Exp, accum_out=sume[:])
                um = sb.tile([128, E], F32, tag="um")
                nc.vector.tensor_tensor(um[:], unn[:], topm[:], op=ALU.mult)
                comb9 = sb.tile([128, 9], F32, tag="c9")
                nc.vector.tensor_scalar_mul(comb9[:, 0:E], um[:], recd[:])
                lnse = sb.tile([128, 1], F32, tag="ls")
                nc.scalar.activation(lnse[:], sume[:], AF.Ln)
                lse = sb.tile([128, 1], F32, tag="le")
                nc.vector.tensor_tensor(lse[:], lnse[:], nm1[:], op=ALU.subtract)
                nc.scalar.activation(comb9[:, 8:9], lse[:], AF.Square)
                c9_ps = ps.tile([9, 128], F32, tag="tc")
                nc.tensor.transpose(c9_ps[:], comb9[:], ident[:])
                c9r = sb.tile([9, 128], F32R, tag="c9r")
                nc.vector.tensor_copy(c9r[:], c9_ps[:])
                cr_ps = ps.tile([1, 128], F32, tag="cr")
                nc.tensor.matmul(cr_ps[:], es_sb[:], c9r[0:E, :],
                                 start=True, stop=True)
                cr_sb = sb.tile([1, 128], F32, tag="crs")
                nc.vector.tensor_copy(cr_sb[:], cr_ps[:])
                nc.sync.dma_start(combrow_d[:, ts(tt, 128)], cr_sb[:])
                nc.sync.dma_start(lsqrow[:, ts(tt, 128)], c9r[8:9, :].bitcast(F32))
            auxacc = sbp.tile([1, 1], F32)
            nc.vector.tensor_reduce(auxacc[:], lsqrow[:],
                                    axis=mybir.AxisListType.X, op=ALU.add)
            aux_sb = sbp.tile([1, 1], F32)
            nc.scalar.activation(aux_sb[:], auxacc[:], AF.Copy, scale=Z_LOSS / T)
            nc.sync.dma_start(aux_out[:], aux_sb[:])

        # ============ Phase 4: MoE (dense, my expert on all tokens) =========
        moe_rs_in = dram.tile([H, T], F32)
        moe_rs_out = dram.tile([RSH, T], F32)
        with ExitStack() as ph:
            sbw = ph.enter_context(tc.tile_pool(name="p4w", bufs=1))
            sb = ph.enter_context(tc.tile_pool(name="p4sb", bufs=2))
            sbh = ph.enter_context(tc.tile_pool(name="p4h", bufs=1))
            psg = ph.enter_context(tc.tile_pool(name="p4g", bufs=2, space="PSUM"))
            psu = ph.enter_context(tc.tile_pool(name="p4u", bufs=2, space="PSUM"))
            psy = ph.enter_context(tc.tile_pool(name="p4y", bufs=2, space="PSUM"))
            psc = ph.enter_context(tc.tile_pool(name="p4c", bufs=1, space="PSUM"))
            wg_sb = sbw.tile([128, KT, II], F32R)
            wu_sb = sbw.tile([128, KT, II], F32R)
            for kk in range(KT):
                nc.sync.dma_start(wg_sb[:, kk, :], r_(wg[ts(kk, 128), :]))
                nc.sync.dma_start(wu_sb[:, kk, :], r_(wu[ts(kk, 128), :]))
            for tci in range(T // 512):
                xn2_sb = sb.tile([128, KT, 512], F32R, tag="x2")
                for kk in range(KT):
                    nc.sync.dma_start(xn2_sb[:, kk, :],
                                      r_(x2nT_flat[ts(kk, 128), ts(tci, 512)]))
                combr_t = sb.tile([1, 512], F32R, tag="cb")
                nc.sync.dma_start(combr_t[:], r_(combrow_d[:, ts(tci, 512)]))
                cw_ps = psc.tile([128, 512], F32, tag="cw")
                nc.tensor.matmul(cw_ps[:], onesr_row[:], combr_t[:],
                                 start=True, stop=True)
                cw_sb = sb.tile([128, 512], F32, tag="cs")
                nc.vector.tensor_copy(cw_sb[:], cw_ps[:])
                hT_sb = sbh.tile([128, ISL, 512], F32R, tag="h")
                for isl in range(ISL):
                    g_ps = psg.tile([128, 512], F32, tag="g")
                    u_ps = psu.tile([128, 512], F32, tag="u")
                    for kk in range(KT):
                        nc.tensor.matmul(g_ps[:], wg_sb[:, kk, ts(isl, 128)],
                                         xn2_sb[:, kk, :],
                                         start=(kk == 0), stop=(kk == KT - 1))
                    for kk in range(KT):
                        nc.tensor.matmul(u_ps[:], wu_sb[:, kk, ts(isl, 128)],
                                         xn2_sb[:, kk, :],
                                         start=(kk == 0), stop=(kk == KT - 1))
                    gs_sb = sb.tile([128, 512], F32, tag="gs")
                    nc.scalar.activation(gs_sb[:], g_ps[:], AF.Silu)
                    nc.vector.tensor_tensor(hT_sb[:, isl, :], gs_sb[:], u_ps[:],
                                            op=ALU.mult)
                for kk in range(KT):
                    wd_sb = sb.tile([128, ISL, 128], F32R, tag="wd")
                    nc.sync.dma_start(
                        wd_sb[:],
                        r_(wd[:, ts(kk, 128)].rearrange("(i p) h -> p i h",
                                                        p=128)))
                    y_ps = psy.tile([128, 512], F32, tag="y")
                    for isl in range(ISL):
                        nc.tensor.matmul(y_ps[:], wd_sb[:, isl, :],
                                         hT_sb[:, isl, :],
                                         start=(isl == 0), stop=(isl == ISL - 1))
                    yw_sb = sb.tile([128, 512], F32, tag="yw")
                    nc.vector.tensor_tensor(yw_sb[:], y_ps[:], cw_sb[:],
                                            op=ALU.mult)
                    nc.sync.dma_start(moe_rs_in[ts(kk, 128), ts(tci, 512)],
                                      yw_sb[:])
        nc.gpsimd.collective_compute(
            "ReduceScatter", ALU.add, replica_groups=[list(range(NC))],
            ins=[moe_rs_in[:].opt()], outs=[moe_rs_out[:].opt()])

        # ============ Phase 5: final residual + output ======================
        with ExitStack() as ph:
            sb = ph.enter_context(tc.tile_pool(name="p5sb", bufs=2))
            for c in range(T // 512):
                mo_t = sb.tile([RSH, 512], F32, tag="mo")
                nc.sync.dma_start(mo_t[:], moe_rs_out[:, ts(c, 512)])
                x1f = sb.tile([RSH, 512], F32, tag="x1f")
                nc.sync.dma_start(x1f[:], x1T_d[:, ts(c, 512)])
                fin_t = sb.tile([RSH, 512], F32, tag="fi")
                nc.vector.tensor_tensor(fin_t[:], mo_t[:], x1f[:],
                                        op=ALU.add)
                nc.sync.dma_start(out_shard[:, ts(c, 512)], fin_t[:])


def _build_nc():
    nc = bacc.Bacc("TRN2", target_bir_lowering=False, debug=False, num_devices=NC)
    _emit(nc)
    nc.compile()
    return nc


def _rope_tables():
    inv = 1.0 / ROPE_BASE ** (np.arange(0, HD, 2, dtype=np.float32) / HD)
    tpos = np.arange(S, dtype=np.float32)
    fr = tpos[:, None] * inv[None, :]
    emb = np.concatenate([fr, fr], axis=1)              # [S, 64]
    cosT = np.ascontiguousarray(np.cos(emb).T).astype(np.float32)
    sinT = np.ascontiguousarray(np.sin(emb).T).astype(np.float32)
    cs1 = np.concatenate([cosT, sinT], axis=0)
    cs2 = np.concatenate([sinT, cosT], axis=0)
    return cs1.astype(np.float32), cs2.astype(np.float32)


def _host_prep(inputs):
    hs = np.ascontiguousarray(inputs["hidden_states"], dtype=np.float32)
    wq = np.asarray(inputs["wq"], np.float32)
    wk = np.asarray(inputs["wk"], np.float32)
    wv = np.asarray(inputs["wv"], np.float32)
    wo = np.asarray(inputs["wo"], np.float32)
    ln1 = np.asarray(inputs["ln1_w"], np.float32)
    ln2 = np.asarray(inputs["ln2_w"], np.float32)
    gate_w = np.asarray(inputs["gate_w"], np.float32)
    w_gate = np.asarray(inputs["w_gate"], np.float32)
    w_up = np.asarray(inputs["w_up"], np.float32)
    w_down = np.asarray(inputs["w_down"], np.float32)

    x = hs.reshape(T, H)
    xT = np.ascontiguousarray(x.T)
    wqf = ln1[:, None] * wq
    wkf = ln1[:, None] * wk
    wvf = ln1[:, None] * wv
    gatef = np.ascontiguousarray(ln2[:, None] * gate_w)
    cs1, cs2 = _rope_tables()

    def rotw(w):
        r = np.empty_like(w)
        r[:, :HD // 2] = -w[:, HD // 2:]
        r[:, HD // 2:] = w[:, :HD // 2]
        return r

    jj = np.arange(896)[None, :]
    pp = np.arange(128)[:, None]
    extm = (jj - 384 - pp >= 0).astype(np.float32)

    in_maps = []
    for c in range(NC):
        b = c // 4
        heads = [3 * (c % 4) + i for i in range(3)]

        def hw(w, h):
            return w[:, HD * h:HD * (h + 1)]

        pq = np.stack([np.concatenate([hw(wqf, h), rotw(hw(wqf, h))], 1)
                       for h in heads]) / np.sqrt(HD)
        pk1 = np.stack([np.concatenate([hw(wkf, h), rotw(hw(wkf, h))], 1)
                        for h in heads])
        pk2 = np.stack([np.concatenate([rotw(hw(wkf, h)), hw(wkf, h)], 1)
                        for h in heads])
        pv_ab = np.concatenate([hw(wvf, heads[0]), hw(wvf, heads[1])], 1)
        pv_c = hw(wvf, heads[2])
        wo3 = np.stack([wo[HD * h:HD * (h + 1), :] for h in heads])
        esel = np.zeros((E, 1), dtype=np.float32)
        esel[c, 0] = 1.0
        am = np.zeros((128, 2), dtype=np.float32)
        am[:, b] = 1.0
        in_maps.append(dict(
            h_shard=np.ascontiguousarray(x[TSH * c:TSH * (c + 1)]),
            hT_rows=np.ascontiguousarray(xT[RSH * c:RSH * (c + 1)]),
            pq=np.ascontiguousarray(pq), pk1=np.ascontiguousarray(pk1),
            pk2=np.ascontiguousarray(pk2),
            pv_ab=np.ascontiguousarray(pv_ab), pv_c=np.ascontiguousarray(pv_c),
            wo3=np.ascontiguousarray(wo3),
            cs1=cs1, cs2=cs2,
            extm=extm, gatew=gatef, esel=esel, amask=am,
            wg=np.ascontiguousarray(ln2[:, None] * w_gate[c]),
            wu=np.ascontiguousarray(ln2[:, None] * w_up[c]),
            wd=np.ascontiguousarray(w_down[c]),
        ))
    return in_maps


def _canonical_mask():
    causal = np.tril(np.ones((S, S), dtype=bool))
    return np.where(causal, 0.0, -1e9).astype(np.float32)[None, None]


def _numpy_fallback(inputs):
    """Faithful numpy port of the reference for non-causal masks."""
    hs = np.asarray(inputs["hidden_states"], np.float64)
    mask = np.asarray(inputs["attention_mask"], np.float64)
    wq, wk, wv, wo = (np.asarray(inputs[k], np.float64)
                      for k in ("wq", "wk", "wv", "wo"))
    ln1, ln2 = (np.asarray(inputs[k], np.float64) for k in ("ln1_w", "ln2_w"))
    gate_w = np.asarray(inputs["gate_w"], np.float64)
    w_gate, w_up, w_down = (np.asarray(inputs[k], np.float64)
                            for k in ("w_gate", "w_up", "w_down"))

    def rms(x, w):
        return w * x / np.sqrt((x * x).mean(-1, keepdims=True) + EPS)

    def rope_cs():
        inv = 1.0 / ROPE_BASE ** (np.arange(0, HD, 2) / HD)
        fr = np.arange(S)[:, None] * inv[None, :]
        emb = np.concatenate([fr, fr], 1)
        return np.cos(emb), np.sin(emb)

    def rot(xv):
        return np.concatenate([-xv[..., HD // 2:], xv[..., :HD // 2]], -1)

    res = hs
    xx = rms(hs, ln1)
    q = (xx @ wq).reshape(B, S, NH, HD).transpose(0, 2, 1, 3)
    k = (xx @ wk).reshape(B, S, NH, HD).transpose(0, 2, 1, 3)
    v = (xx @ wv).reshape(B, S, NH, HD).transpose(0, 2, 1, 3)
    cos, sin = rope_cs()
    q = q * cos + rot(q) * sin
    k = k * cos + rot(k) * sin
    sc = np.einsum('bhqd,bhkd->bhqk', q, k) / np.sqrt(HD) + mask
    sc = sc - sc.max(-1, keepdims=True)
    p = np.exp(sc)
    p /= p.sum(-1, keepdims=True)
    o = np.einsum('bhqk,bhkd->bhqd', p, v).transpose(0, 2, 1, 3).reshape(B, S, H)
    x1 = res + o @ wo
    xn = rms(x1, ln2)
    xt = xn.reshape(-1, H)
    lg = xt @ gate_w
    lg_s = lg - lg.max(-1, keepdims=True)
    pr = np.exp(lg_s)
    pr /= pr.sum(-1, keepdims=True)
    idx = np.argsort(-pr, axis=-1)[:, :2]
    tw = np.take_along_axis(pr, idx, -1)
    tw /= tw.sum(-1, keepdims=True)
    comb = np.zeros_like(pr)
    np.put_along_axis(comb, idx, tw, -1)
    out = np.zeros_like(xt)
    for e in range(E):
        h = xt @ w_gate[e]
        h = h / (1 + np.exp(-h)) * (xt @ w_up[e])
        out += comb[:, e:e + 1] * (h @ w_down[e])
    lse = np.log(np.exp(lg - lg.max(-1, keepdims=True)).sum(-1)) + lg.max(-1)
    aux = Z_LOSS * np.mean(lse ** 2)
    return ((x1 + out.reshape(B, S, H)).astype(np.float32),
            np.float32(aux))


def kernel(**inputs):
    mask = np.asarray(inputs["attention_mask"], np.float32)
    if not np.array_equal(mask, _canonical_mask()):
        return _numpy_fallback(inputs)
    if "nc" not in _CACHE:
        _CACHE["nc"] = _build_nc()
    in_maps = _host_prep(inputs)
    res = run_bass_kernel_spmd(_CACHE["nc"], in_maps, list(range(NC))).results
    outT = np.concatenate([res[c]["out_shard"] for c in range(NC)], axis=0)
    out = np.ascontiguousarray(outT.T).reshape(B, S, H)
    aux = np.float32(res[0]["aux_out"][0, 0])
    return out, aux


# revision 9
# speedup vs baseline: 2178.3336x; 2178.3336x over previous
"""Trainium2 Bass kernel for nn_DecoderLayer_88424786690751 (MoE decoder layer).

8-core SPMD strategy:
  - ln1/ln2 folded into consuming weights host-side; RoPE folded into augmented
    projection weights: scores accumulate two K=128 matmuls
    (kaug1.T@qaug + kaug2.T@qaug == rope(k) @ rope(q)^T exactly).
  - Attention: 24 (batch, head) jobs -> 3 heads/core, cores 0-3 batch 0,
    cores 4-7 batch 1. Transposed activations [d, t]. Causal block skipping;
    diagonal blocks masked multiplicatively post-exp (host-passed 0/1 tiles).
    Softmax denominator from a ones column stacked into v (vaug).
    Batch selection is pure data: group AllGather [[0-3],[4-7]] feeds each
    core its batch's tokens at identical offsets; wo outputs are written to
    both batch column-halves scaled by a per-core 0/1 alpha mask so a single
    global ReduceScatter(add) sums partials correctly.
  - Router replicated over all tokens (tiny); per-core expert row selected by
    a one-hot matmul. MoE dense expert-parallel: core e computes expert e on
    all tokens, scaled by its routing-weight column (0 for unrouted tokens,
    so this matches top-2 routing exactly).
  - Two global ReduceScatters [768,4096]->[96,4096] (attention, MoE); host
    concatenates the 8 row-shards and transposes back.
All matmuls fp32r (full PE rate at N>=256, ~2e-4 relative precision).
"""
import sys
if '/opt/trn_rl_repo' not in sys.path:
    sys.path.insert(0, '/opt/trn_rl_repo')

from contextlib import ExitStack

import numpy as np

import concourse.bass as bass
import concourse.mybir as mybir
import concourse.tile as tile
from concourse import bacc
from concourse.bass_utils import run_bass_kernel_spmd
from concourse.masks import make_identity

F32 = mybir.dt.float32
F32R = mybir.dt.float32r
AF = mybir.ActivationFunctionType
ALU = mybir.AluOpType
ts = bass.ts

H, NH, HD, II, E = 768, 12, 64, 2048, 8
B, S = 2, 2048
T = B * S          # 4096
NC = 8
EPS = 1e-6
Z_LOSS = 0.001
ROPE_BASE = 10000.0
KT = H // 128       # 6   H k-tiles
ISL = II // 128     # 16  I-slices
QCH = S // 512      # 4   per-batch 512-token chunks
SKT = S // 128      # 16  per-batch 128-token key tiles
RSH = H // NC       # 96  H-row shard
TSH = T // NC       # 512 token shard

_CACHE = {}


def _emit(nc):
    din = lambda n, s: nc.declare_dram_parameter(n, list(s), F32, isOutput=False)
    h_shard = din("h_shard", (TSH, H))
    hT_rows = din("hT_rows", (RSH, T))
    pq_in = din("pq", (3, H, 128))
    pk1_in = din("pk1", (3, H, 128))
    pk2_in = din("pk2", (3, H, 128))
    pv_ab = din("pv_ab", (H, 128))
    pv_c = din("pv_c", (H, HD))
    wo3 = din("wo3", (3, HD, H))
    cs1 = din("cs1", (128, S))
    cs2 = din("cs2", (128, S))
    extm = din("extm", (128, 896))
    gatew = din("gatew", (H, E))
    esel = din("esel", (E, 1))
    amask = din("amask", (128, 2))
    wg = din("wg", (H, II))
    wu = din("wu", (H, II))
    wd = din("wd", (II, H))
    out_shard = nc.declare_dram_parameter("out_shard", [RSH, T], F32, isOutput=True)
    aux_out = nc.declare_dram_parameter("aux_out", [1, 1], F32, isOutput=True)

    r_ = lambda ap: ap.bitcast(F32R)

    with tile.TileContext(nc) as tc, ExitStack() as top:
        pers = top.enter_context(tc.tile_pool(name="pers", bufs=1))
        dram = top.enter_context(tc.tile_pool(name="dram", bufs=1, space="DRAM"))

        ident = pers.tile([128, 128], F32)
        make_identity(nc, ident[:])
        ones_f = pers.tile([128, 1], F32)
        nc.vector.memset(ones_f[:], 1.0)
        onesr_col = pers.tile([128, 1], F32R)       # ones lhsT [K<=128, 1]
        nc.vector.tensor_copy(onesr_col[:], ones_f[:])
        ones_fr = pers.tile([1, 128], F32)
        nc.vector.memset(ones_fr[:], 1.0)
        onesr_row = pers.tile([1, 128], F32R)       # ones lhsT [1, M<=128]
        nc.vector.tensor_copy(onesr_row[:], ones_fr[:])
        eps_sb = pers.tile([128, 1], F32)
        nc.vector.memset(eps_sb[:], EPS)
        am_sb = pers.tile([128, 2], F32)
        nc.sync.dma_start(am_sb[:], amask[:])

        combrow_d = dram.tile([1, T], F32)          # my expert's comb weights
        x1T_d = dram.tile([RSH, T], F32)            # hidden+attn rows

        # ============ Phase 0: ln1 rmsnorm + transpose + group AllGather ====
        xnT_shard_d = dram.tile([H, TSH], F32)
        xnT_ag = dram.tile([4, H, TSH], F32)
        with ExitStack() as ph:
            sb = ph.enter_context(tc.tile_pool(name="p0sb", bufs=2))
            ps = ph.enter_context(tc.tile_pool(name="p0ps", bufs=2, space="PSUM"))
            for j in range(TSH // 128):
                x_t = sb.tile([128, H], F32, tag="x")
                nc.sync.dma_start(x_t[:], h_shard[ts(j, 128), :])
                sq_t = sb.tile([128, H], F32, tag="sq")
                ss_t = sb.tile([128, 1], F32, tag="ss")
                nc.scalar.activation(sq_t[:], x_t[:], AF.Square, accum_out=ss_t[:])
                std_t = sb.tile([128, 1], F32, tag="sd")
                nc.scalar.activation(std_t[:], ss_t[:], AF.Sqrt,
                                     bias=eps_sb[:], scale=1.0 / H)
                rcp_t = sb.tile([128, 1], F32, tag="rc")
                nc.vector.reciprocal(rcp_t[:], std_t[:])
                xn_t = sb.tile([128, H], F32, tag="xn")
                nc.vector.tensor_scalar_mul(xn_t[:], x_t[:], rcp_t[:])
                for kk in range(KT):
                    tr_ps = ps.tile([128, 128], F32, tag="tr")
                    nc.tensor.transpose(tr_ps[:], xn_t[:, ts(kk, 128)], ident[:])
                    tr_sb = sb.tile([128, 128], F32, tag="tb")
                    nc.vector.tensor_copy(tr_sb[:], tr_ps[:])
                    nc.sync.dma_start(
                        xnT_shard_d[ts(kk, 128), ts(j, 128)], tr_sb[:])
        nc.gpsimd.collective_compute(
            "AllGather", ALU.bypass,
            replica_groups=[[0, 1, 2, 3], [4, 5, 6, 7]],
            ins=[xnT_shard_d[:].opt()], outs=[xnT_ag[:].opt()])

        # ============ Phase 1: attention ====================================
        attn_rs_in = dram.tile([H, T], F32)
        attn_rs_out = dram.tile([RSH, T], F32)
        with ExitStack() as ph:
            sbw = ph.enter_context(tc.tile_pool(name="p1w", bufs=1))
            sb = ph.enter_context(tc.tile_pool(name="p1sb", bufs=2))
            ps = ph.enter_context(tc.tile_pool(name="p1ps", bufs=2, space="PSUM"))
            psav = ph.enter_context(tc.tile_pool(name="p1av", bufs=2, space="PSUM"))
            pswo = ph.enter_context(tc.tile_pool(name="p1wo", bufs=2, space="PSUM"))
            psbc = ph.enter_context(tc.tile_pool(name="p1bc", bufs=1, space="PSUM"))

            xq_sb = sbw.tile([128, KT, QCH, 512], F32R)   # my batch's xnT
            for kk in range(KT):
                for q in range(QCH):
                    nc.sync.dma_start(xq_sb[:, kk, q, :],
                                      r_(xnT_ag[q, ts(kk, 128), :]))
            pvab_sb = sbw.tile([128, KT, 128], F32R)
            pvc_sb = sbw.tile([128, KT, HD], F32R)
            for kk in range(KT):
                nc.sync.dma_start(pvab_sb[:, kk, :], r_(pv_ab[ts(kk, 128), :]))
                nc.sync.dma_start(pvc_sb[:, kk, :], r_(pv_c[ts(kk, 128), :]))
            wo_sb = sbw.tile([HD, 3, KT, 128], F32R)
            for i in range(3):
                for kk in range(KT):
                    nc.sync.dma_start(wo_sb[:, i, kk, :], r_(wo3[i, :, ts(kk, 128)]))
            cs1_sb = sbw.tile([128, S], F32R)
            cs2_sb = sbw.tile([128, S], F32R)
            nc.sync.dma_start(cs1_sb[:], r_(cs1[:]))
            nc.sync.dma_start(cs2_sb[:], r_(cs2[:]))
            dm_sb = sbw.tile([128, 896], F32R)
            nc.sync.dma_start(dm_sb[:], r_(extm[:]))

            # --- v for all 3 heads, then vaug = [v | 1] transposed back ---
            vaug_sb = sbw.tile([128, SKT, 3, 65], F32R)
            for q in range(QCH):
                vab_ps = ps.tile([128, 512], F32, tag="sc")
                for kk in range(KT):
                    nc.tensor.matmul(vab_ps[:], pvab_sb[:, kk, :],
                                     xq_sb[:, kk, q, :],
                                     start=(kk == 0), stop=(kk == KT - 1))
                vabq = sb.tile([128, 512], F32, tag="vab")
                nc.vector.tensor_copy(vabq[:], vab_ps[:])
                vc_ps = ps.tile([HD, 512], F32, tag="sc")
                for kk in range(KT):
                    nc.tensor.matmul(vc_ps[:], pvc_sb[:, kk, :],
                                     xq_sb[:, kk, q, :],
                                     start=(kk == 0), stop=(kk == KT - 1))
                vcq = sb.tile([HD, 512], F32, tag="vc")
                nc.vector.tensor_copy(vcq[:], vc_ps[:])
                for sub in range(4):
                    kt = 4 * q + sub
                    trab_ps = psav.tile([128, 128], F32, tag="av")
                    nc.tensor.transpose(trab_ps[:], vabq[:, ts(sub, 128)],
                                        ident[:])
                    nc.vector.tensor_copy(vaug_sb[:, kt, 0, 0:64],
                                          trab_ps[:, 0:64])
                    nc.vector.tensor_copy(vaug_sb[:, kt, 1, 0:64],
                                          trab_ps[:, 64:128])
                    trc_ps = psav.tile([128, 64], F32, tag="av")
                    nc.tensor.transpose(trc_ps[:], vcq[:, ts(sub, 128)],
                                        ident[0:64, 0:64])
                    nc.vector.tensor_copy(vaug_sb[:, kt, 2, 0:64], trc_ps[:])
                    for i in range(3):
                        nc.vector.tensor_copy(vaug_sb[:, kt, i, 64:65],
                                              ones_f[:])

            # --- per pair: augmented projections, then scores/av/normalize ---
            outT_st = sbw.tile([HD, 3, QCH, 512], F32R)   # normalized heads
            for i in range(3):
                pqi_sb = sbw.tile([128, KT, 128], F32R, tag="pqi")
                pk1i_sb = sbw.tile([128, KT, 128], F32R, tag="pk1i")
                pk2i_sb = sbw.tile([128, KT, 128], F32R, tag="pk2i")
                for kk in range(KT):
                    nc.sync.dma_start(pqi_sb[:, kk, :], r_(pq_in[i, ts(kk, 128), :]))
                    nc.sync.dma_start(pk1i_sb[:, kk, :], r_(pk1_in[i, ts(kk, 128), :]))
                    nc.sync.dma_start(pk2i_sb[:, kk, :], r_(pk2_in[i, ts(kk, 128), :]))
                qaug_sb = sbw.tile([128, S], F32R, tag="qaug")
                kaug1_sb = sbw.tile([128, S], F32R, tag="kaug1")
                kaug2_sb = sbw.tile([128, S], F32R, tag="kaug2")
                for q in range(QCH):
                    for (w_sb, cs_sb, dst) in (
                        (pqi_sb, cs1_sb, qaug_sb),
                        (pk1i_sb, cs1_sb, kaug1_sb),
                        (pk2i_sb, cs2_sb, kaug2_sb),
                    ):
                        pr_ps = ps.tile([128, 512], F32, tag="sc")
                        for kk in range(KT):
                            nc.tensor.matmul(pr_ps[:], w_sb[:, kk, :],
                                             xq_sb[:, kk, q, :],
                                             start=(kk == 0), stop=(kk == KT - 1))
                        nc.vector.tensor_tensor(
                            dst[:, ts(q, 512)], pr_ps[:],
                            cs_sb[:, ts(q, 512)], op=ALU.mult)
                for q in range(QCH):
                    nkt = 4 * q + 4
                    av_ps = psav.tile([65, 512], F32, tag="av")
                    for kt in range(nkt):
                        sc_ps = ps.tile([128, 512], F32, tag="sc")
                        nc.tensor.matmul(sc_ps[:], kaug1_sb[:, ts(kt, 128)],
                                         qaug_sb[:, ts(q, 512)],
                                         start=True, stop=False)
                        nc.tensor.matmul(sc_ps[:], kaug2_sb[:, ts(kt, 128)],
                                         qaug_sb[:, ts(q, 512)],
                                         start=False, stop=True)
                        pr_sb = sb.tile([128, 512], F32R, tag="pr")
                        nc.scalar.activation(pr_sb[:], sc_ps[:], AF.Exp)
                        m = kt - 4 * q
                        if m >= 0:
                            nc.vector.tensor_tensor(
                                pr_sb[:], pr_sb[:],
                                dm_sb[:, bass.ds((3 - m) * 128, 512)],
                                op=ALU.mult)
                        nc.tensor.matmul(av_ps[:], vaug_sb[:, kt, i, :], pr_sb[:],
                                         start=(kt == 0), stop=(kt == nkt - 1))
                    o_sb = sb.tile([65, 512], F32, tag="o")
                    nc.vector.tensor_copy(o_sb[:], av_ps[:])
                    den_sb = sb.tile([1, 512], F32, tag="dn")
                    nc.sync.dma_start(den_sb[:], o_sb[64:65, :])
                    rcp_sb = sb.tile([1, 512], F32, tag="rcd")
                    nc.vector.reciprocal(rcp_sb[:], den_sb[:])
                    rcpr_sb = sb.tile([1, 512], F32R, tag="rcr")
                    nc.vector.tensor_copy(rcpr_sb[:], rcp_sb[:])
                    bc_ps = psbc.tile([64, 512], F32, tag="bc")
                    nc.tensor.matmul(bc_ps[:], onesr_row[:, 0:64], rcpr_sb[:],
                                     start=True, stop=True)
                    nc.vector.tensor_tensor(outT_st[:, i, q, :], o_sb[0:64, :],
                                            bc_ps[:], op=ALU.mult)
            # --- wo projection, alpha-masked double write ---
            for q in range(QCH):
                for kk in range(KT):
                    wo_ps = pswo.tile([128, 512], F32, tag="wo")
                    for i in range(3):
                        nc.tensor.matmul(wo_ps[:], wo_sb[:, i, kk, :],
                                         outT_st[:, i, q, :],
                                         start=(i == 0), stop=(i == 2))
                    for bb in range(2):
                        aw_sb = sb.tile([128, 512], F32, tag="aw")
                        nc.vector.tensor_scalar_mul(aw_sb[:], wo_ps[:],
                                                    am_sb[:, bb:bb + 1])
                        nc.sync.dma_start(
                            attn_rs_in[ts(kk, 128),
                                       bass.ds(2048 * bb + 512 * q, 512)],
                            aw_sb[:])
        nc.gpsimd.collective_compute(
            "ReduceScatter", ALU.add, replica_groups=[list(range(NC))],
            ins=[attn_rs_in[:].opt()], outs=[attn_rs_out[:].opt()])

        # ============ Phase 2: residual + ln2 on [96, 4096] row shard =======
        x2n_shard_d = dram.tile([RSH, T], F32)
        x2nT_ag = dram.tile([NC, RSH, T], F32, addr_space="Shared")
        ss_d = dram.tile([1, T], F32)
        ss_ag = dram.tile([NC, 1, T], F32, addr_space="Shared")
        with ExitStack() as ph:
            sb = ph.enter_context(tc.tile_pool(name="p2sb", bufs=2))
            sbp = ph.enter_context(tc.tile_pool(name="p2sbp", bufs=1))
            ps = ph.enter_context(tc.tile_pool(name="p2ps", bufs=2, space="PSUM"))
            ss_sb = sbp.tile([1, T], F32)
            for c in range(T // 512):
                ar_t = sb.tile([RSH, 512], F32, tag="ar")
                nc.sync.dma_start(ar_t[:], attn_rs_out[:, ts(c, 512)])
                ht_t = sb.tile([RSH, 512], F32, tag="ht")
                nc.sync.dma_start(ht_t[:], hT_rows[:, ts(c, 512)])
                x1c = sb.tile([RSH, 512], F32, tag="x1")
                nc.vector.tensor_tensor(x1c[:], ar_t[:], ht_t[:], op=ALU.add)
                nc.sync.dma_start(x1T_d[:, ts(c, 512)], x1c[:])
                sq_t = sb.tile([RSH, 512], F32R, tag="sq")
                nc.scalar.activation(sq_t[:], x1c[:], AF.Square)
                ssp_ps = ps.tile([1, 512], F32, tag="ss")
                nc.tensor.matmul(ssp_ps[:], onesr_col[0:RSH, :], sq_t[:],
                                 start=True, stop=True)
                nc.vector.tensor_copy(ss_sb[:, ts(c, 512)], ssp_ps[:])
            nc.sync.dma_start(ss_d[:], ss_sb[:])
            nc.gpsimd.collective_compute(
                "AllGather", ALU.bypass, replica_groups=[list(range(NC))],
                ins=[ss_d[:].opt()], outs=[ss_ag[:].opt()])
            ssall_sb = sbp.tile([NC, T], F32R)
            nc.sync.dma_start(ssall_sb[:],
                              r_(ss_ag[:].rearrange("r a t -> (r a) t")))
            rs2_sb = sbp.tile([1, T], F32)
            rs2r_sb = sbp.tile([1, T], F32R)
            for c in range(T // 512):
                var_ps = ps.tile([1, 512], F32, tag="ss")
                nc.tensor.matmul(var_ps[:], onesr_col[0:NC, :],
                                 ssall_sb[:, ts(c, 512)], start=True, stop=True)
                std_t = sb.tile([1, 512], F32, tag="sd")
                nc.scalar.activation(std_t[:], var_ps[:], AF.Sqrt,
                                     bias=eps_sb[0:1, :], scale=1.0 / H)
                nc.vector.reciprocal(rs2_sb[:, ts(c, 512)], std_t[:])
            nc.vector.tensor_copy(rs2r_sb[:], rs2_sb[:])
            for c in range(T // 512):
                bc_ps = ps.tile([RSH, 512], F32, tag="bc")
                nc.tensor.matmul(bc_ps[:], onesr_row[:, 0:RSH],
                                 rs2r_sb[:, ts(c, 512)], start=True, stop=True)
                x1b = sb.tile([RSH, 512], F32, tag="x1b")
                nc.sync.dma_start(x1b[:], x1T_d[:, ts(c, 512)])
                x2n_t = sb.tile([RSH, 512], F32, tag="x2")
                nc.vector.tensor_tensor(x2n_t[:], x1b[:], bc_ps[:],
                                        op=ALU.mult)
                nc.sync.dma_start(x2n_shard_d[:, ts(c, 512)], x2n_t[:])
        nc.gpsimd.collective_compute(
            "AllGather", ALU.bypass, replica_groups=[list(range(NC))],
            ins=[x2n_shard_d[:].opt()], outs=[x2nT_ag[:].opt()])
        x2nT_flat = x2nT_ag[:].rearrange("r p t -> (r p) t")  # [768, 4096]

        # ============ Phase 3: router (replicated over all tokens) ==========
        with ExitStack() as ph:
            sb = ph.enter_context(tc.tile_pool(name="p3sb", bufs=2))
            sbp = ph.enter_context(tc.tile_pool(name="p3sbp", bufs=1))
            ps = ph.enter_context(tc.tile_pool(name="p3ps", bufs=2, space="PSUM"))
            gw_sb = sbp.tile([128, KT, E], F32)
            for kk in range(KT):
                nc.sync.dma_start(gw_sb[:, kk, :], gatew[ts(kk, 128), :])
            es_sb = sbp.tile([E, 1], F32R)
            nc.sync.dma_start(es_sb[:], r_(esel[:]))
            lsqrow = sbp.tile([1, T], F32)
            for tt in range(T // 128):
                xt_sb = sb.tile([128, KT, 128], F32, tag="xt")
                for kk in range(KT):
                    nc.sync.dma_start(xt_sb[:, kk, :],
                                      x2nT_flat[ts(kk, 128), ts(tt, 128)])
                lg_ps = ps.tile([128, E], F32, tag="lg")
                for kk in range(KT):
                    nc.tensor.matmul(lg_ps[:], xt_sb[:, kk, :], gw_sb[:, kk, :],
                                     start=(kk == 0), stop=(kk == KT - 1))
                lg = sb.tile([128, E], F32, tag="l0")
                nc.vector.tensor_copy(lg[:], lg_ps[:])
                nm1 = sb.tile([128, 1], F32, tag="m1")
                nc.vector.tensor_reduce(nm1[:], lg[:], axis=mybir.AxisListType.X,
                                        op=ALU.max, negate=True)
                t1 = sb.tile([128, E], F32, tag="t1")
                nc.vector.tensor_scalar_add(t1[:], lg[:], nm1[:])
                is1 = sb.tile([128, E], F32, tag="i1")
                nc.vector.tensor_scalar(is1[:], t1[:], 0.0, None, op0=ALU.is_ge)
                big = sb.tile([128, E], F32, tag="bg")
                nc.vector.tensor_scalar_mul(big[:], is1[:], -1e9)
                t2 = sb.tile([128, E], F32, tag="t2")
                nc.vector.tensor_tensor(t2[:], t1[:], big[:], op=ALU.add)
                nm2 = sb.tile([128, 1], F32, tag="m2")
                nc.vector.tensor_reduce(nm2[:], t2[:], axis=mybir.AxisListType.X,
                                        op=ALU.max, negate=True)
                iz = sb.tile([128, E], F32, tag="iz")
                nc.vector.tensor_scalar_add(iz[:], t2[:], nm2[:])
                is2 = sb.tile([128, E], F32, tag="i2")
                nc.vector.tensor_scalar(is2[:], iz[:], 0.0, None, op0=ALU.is_ge)
                topm = sb.tile([128, E], F32, tag="tm")
                nc.vector.tensor_tensor(topm[:], is1[:], is2[:], op=ALU.add)
                em2 = sb.tile([128, 1], F32, tag="e2")
                nc.scalar.activation(em2[:], nm2[:], AF.Exp, scale=-1.0)
                den2 = sb.tile([128, 1], F32, tag="d2")
                nc.vector.tensor_scalar_add(den2[:], em2[:], 1.0)
                recd = sb.tile([128, 1], F32, tag="rd")
                nc.vector.reciprocal(recd[:], den2[:])
                unn = sb.tile([128, E], F32, tag="un")
                sume = sb.tile([128, 1], F32, tag="se")
                nc.scalar.activation(unn[:], t1[:], AF.Exp, accum_out=sume[:])
                um = sb.tile([128, E], F32, tag="um")
                nc.vector.tensor_tensor(um[:], unn[:], topm[:], op=ALU.mult)
                comb9 = sb.tile([128, 9], F32, tag="c9")
                nc.vector.tensor_scalar_mul(comb9[:, 0:E], um[:], recd[:])
                lnse = sb.tile([128, 1], F32, tag="ls")
                nc.scalar.activation(lnse[:], sume[:], AF.Ln)
                lse = sb.tile([128, 1], F32, tag="le")
                nc.vector.tensor_tensor(lse[:], lnse[:], nm1[:], op=ALU.subtract)
                nc.scalar.activation(comb9[:, 8:9], lse[:], AF.Square)
                c9_ps = ps.tile([9, 128], F32, tag="tc")
                nc.tensor.transpose(c9_ps[:], comb9[:], ident[:])
                c9r = sb.tile([9, 128], F32R, tag="c9r")
                nc.vector.tensor_copy(c9r[:], c9_ps[:])
                cr_ps = ps.tile([1, 128], F32, tag="cr")
                nc.tensor.matmul(cr_ps[:], es_sb[:], c9r[0:E, :],
                                 start=True, stop=True)
                cr_sb = sb.tile([1, 128], F32, tag="crs")
                nc.vector.tensor_copy(cr_sb[:], cr_ps[:])
                nc.sync.dma_start(combrow_d[:, ts(tt, 128)], cr_sb[:])
                nc.sync.dma_start(lsqrow[:, ts(tt, 128)], c9r[8:9, :].bitcast(F32))
            auxacc = sbp.tile([1, 1], F32)
            nc.vector.tensor_reduce(auxacc[:], lsqrow[:],
                                    axis=mybir.AxisListType.X, op=ALU.add)
            aux_sb = sbp.tile([1, 1], F32)
            nc.scalar.activation(aux_sb[:], auxacc[:], AF.Copy, scale=Z_LOSS / T)
            nc.sync.dma_start(aux_out[:], aux_sb[:])

        # ============ Phase 4: MoE (dense, my expert on all tokens) =========
        moe_rs_in = dram.tile([H, T], F32)
        moe_rs_out = dram.tile([RSH, T], F32)
        with ExitStack() as ph:
            sbw = ph.enter_context(tc.tile_pool(name="p4w", bufs=1))
            sb = ph.enter_context(tc.tile_pool(name="p4sb", bufs=2))
            sbh = ph.enter_context(tc.tile_pool(name="p4h", bufs=1))
            psg = ph.enter_context(tc.tile_pool(name="p4g", bufs=2, space="PSUM"))
            psu = ph.enter_context(tc.tile_pool(name="p4u", bufs=2, space="PSUM"))
            psy = ph.enter_context(tc.tile_pool(name="p4y", bufs=2, space="PSUM"))
            psc = ph.enter_context(tc.tile_pool(name="p4c", bufs=1, space="PSUM"))
            wg_sb = sbw.tile([128, KT, II], F32R)
            wu_sb = sbw.tile([128, KT, II], F32R)
            for kk in range(KT):
                nc.sync.dma_start(wg_sb[:, kk, :], r_(wg[ts(kk, 128), :]))
                nc.sync.dma_start(wu_sb[:, kk, :], r_(wu[ts(kk, 128), :]))
            for tci in range(T // 512):
                xn2_sb = sb.tile([128, KT, 512], F32R, tag="x2")
                for kk in range(KT):
                    nc.sync.dma_start(xn2_sb[:, kk, :],
                                      r_(x2nT_flat[ts(kk, 128), ts(tci, 512)]))
                combr_t = sb.tile([1, 512], F32R, tag="cb")
                nc.sync.dma_start(combr_t[:], r_(combrow_d[:, ts(tci, 512)]))
                cw_ps = psc.tile([128, 512], F32, tag="cw")
                nc.tensor.matmul(cw_ps[:], onesr_row[:], combr_t[:],
                                 start=True, stop=True)
                cw_sb = sb.tile([128, 512], F32, tag="cs")
                nc.vector.tensor_copy(cw_sb[:], cw_ps[:])
                hT_sb = sbh.tile([128, ISL, 512], F32R, tag="h")
                for isl in range(ISL):
                    g_ps = psg.tile([128, 512], F32, tag="g")
                    u_ps = psu.tile([128, 512], F32, tag="u")
                    for kk in range(KT):
                        nc.tensor.matmul(g_ps[:], wg_sb[:, kk, ts(isl, 128)],
                                         xn2_sb[:, kk, :],
                                         start=(kk == 0), stop=(kk == KT - 1))
                    for kk in range(KT):
                        nc.tensor.matmul(u_ps[:], wu_sb[:, kk, ts(isl, 128)],
                                         xn2_sb[:, kk, :],
                                         start=(kk == 0), stop=(kk == KT - 1))
                    gs_sb = sb.tile([128, 512], F32, tag="gs")
                    nc.scalar.activation(gs_sb[:], g_ps[:], AF.Silu)
                    nc.vector.tensor_tensor(hT_sb[:, isl, :], gs_sb[:], u_ps[:],
                                            op=ALU.mult)
                for kk in range(KT):
                    wd_sb = sb.tile([128, ISL, 128], F32R, tag="wd")
                    nc.sync.dma_start(
                        wd_sb[:],
                        r_(wd[:, ts(kk, 128)].rearrange("(i p) h -> p i h",
                                                        p=128)))
                    y_ps = psy.tile([128, 512], F32, tag="y")
                    for isl in range(ISL):
                        nc.tensor.matmul(y_ps[:], wd_sb[:, isl, :],
                                         hT_sb[:, isl, :],
                                         start=(isl == 0), stop=(isl == ISL - 1))
                    yw_sb = sb.tile([128, 512], F32, tag="yw")
                    nc.vector.tensor_tensor(yw_sb[:], y_ps[:], cw_sb[:],
                                            op=ALU.mult)
                    nc.sync.dma_start(moe_rs_in[ts(kk, 128), ts(tci, 512)],
                                      yw_sb[:])
        nc.gpsimd.collective_compute(
            "ReduceScatter", ALU.add, replica_groups=[list(range(NC))],
            ins=[moe_rs_in[:].opt()], outs=[moe_rs_out[:].opt()])

        # ============ Phase 5: final residual + output ======================
        with ExitStack() as ph:
            sb = ph.enter_context(tc.tile_pool(name="p5sb", bufs=2))
            for c in range(T // 512):
                mo_t = sb.tile([RSH, 512], F32, tag="mo")
                nc.sync.dma_start(mo_t[:], moe_rs_out[:, ts(c, 512)])
                x1f = sb.tile([RSH, 512], F32, tag="x1f")
                nc.sync.dma_start(x1f[:], x1T_d[:, ts(c, 512)])
                fin_t = sb.tile([RSH, 512], F32, tag="fi")
                nc.vector.tensor_tensor(fin_t[:], mo_t[:], x1f[:],
                                        op=ALU.add)
                nc.sync.dma_start(out_shard[:, ts(c, 512)], fin_t[:])


def _build_nc():
    nc = bacc.Bacc("TRN2", target_bir_lowering=False, debug=False, num_devices=NC)
    _emit(nc)
    nc.compile()
    return nc


def _rope_tables():
    inv = 1.0 / ROPE_BASE ** (np.arange(0, HD, 2, dtype=np.float32) / HD)
    tpos = np.arange(S, dtype=np.float32)
    fr = tpos[:, None] * inv[None, :]
    emb = np.concatenate([fr, fr], axis=1)              # [S, 64]
    cosT = np.ascontiguousarray(np.cos(emb).T).astype(np.float32)
    sinT = np.ascontiguousarray(np.sin(emb).T).astype(np.float32)
    cs1 = np.concatenate([cosT, sinT], axis=0)
    cs2 = np.concatenate([sinT, cosT], axis=0)
    return cs1.astype(np.float32), cs2.astype(np.float32)


def _host_prep(inputs):
    hs = np.ascontiguousarray(inputs["hidden_states"], dtype=np.float32)
    wq = np.asarray(inputs["wq"], np.float32)
    wk = np.asarray(inputs["wk"], np.float32)
    wv = np.asarray(inputs["wv"], np.float32)
    wo = np.asarray(inputs["wo"], np.float32)
    ln1 = np.asarray(inputs["ln1_w"], np.float32)
    ln2 = np.asarray(inputs["ln2_w"], np.float32)
    gate_w = np.asarray(inputs["gate_w"], np.float32)
    w_gate = np.asarray(inputs["w_gate"], np.float32)
    w_up = np.asarray(inputs["w_up"], np.float32)
    w_down = np.asarray(inputs["w_down"], np.float32)

    x = hs.reshape(T, H)
    xT = np.ascontiguousarray(x.T)
    wqf = ln1[:, None] * wq
    wkf = ln1[:, None] * wk
    wvf = ln1[:, None] * wv
    gatef = np.ascontiguousarray(ln2[:, None] * gate_w)
    cs1, cs2 = _rope_tables()

    def rotw(w):
        r = np.empty_like(w)
        r[:, :HD // 2] = -w[:, HD // 2:]
        r[:, HD // 2:] = w[:, :HD // 2]
        return r

    jj = np.arange(896)[None, :]
    pp = np.arange(128)[:, None]
    extm = (jj - 384 - pp >= 0).astype(np.float32)

    in_maps = []
    for c in range(NC):
        b = c // 4
        heads = [3 * (c % 4) + i for i in range(3)]

        def hw(w, h):
            return w[:, HD * h:HD * (h + 1)]

        pq = np.stack([np.concatenate([hw(wqf, h), rotw(hw(wqf, h))], 1)
                       for h in heads]) / np.sqrt(HD)
        pk1 = np.stack([np.concatenate([hw(wkf, h), rotw(hw(wkf, h))], 1)
                        for h in heads])
        pk2 = np.stack([np.concatenate([rotw(hw(wkf, h)), hw(wkf, h)], 1)
                        for h in heads])
        pv_ab = np.concatenate([hw(wvf, heads[0]), hw(wvf, heads[1])], 1)
        pv_c = hw(wvf, heads[2])
        wo3 = np.stack([wo[HD * h:HD * (h + 1), :] for h in heads])
        esel = np.zeros((E, 1), dtype=np.float32)
        esel[c, 0] = 1.0
        am = np.zeros((128, 2), dtype=np.float32)
        am[:, b] = 1.0
        in_maps.append(dict(
            h_shard=np.ascontiguousarray(x[TSH * c:TSH * (c + 1)]),
            hT_rows=np.ascontiguousarray(xT[RSH * c:RSH * (c + 1)]),
            pq=np.ascontiguousarray(pq), pk1=np.ascontiguousarray(pk1),
            pk2=np.ascontiguousarray(pk2),
            pv_ab=np.ascontiguousarray(pv_ab), pv_c=np.ascontiguousarray(pv_c),
            wo3=np.ascontiguousarray(wo3),
            cs1=cs1, cs2=cs2,
            extm=extm, gatew=gatef, esel=esel, amask=am,
            wg=np.ascontiguousarray(ln2[:, None] * w_gate[c]),
            wu=np.ascontiguousarray(ln2[:, None] * w_up[c]),
            wd=np.ascontiguousarray(w_down[c]),
        ))
    return in_maps


def _canonical_mask():
    causal = np.tril(np.ones((S, S), dtype=bool))
    return np.where(causal, 0.0, -1e9).astype(np.float32)[None, None]


def _numpy_fallback(inputs):
    """Faithful numpy port of the reference for non-causal masks."""
    hs = np.asarray(inputs["hidden_states"], np.float64)
    mask = np.asarray(inputs["attention_mask"], np.float64)
    wq, wk, wv, wo = (np.asarray(inputs[k], np.float64)
                      for k in ("wq", "wk", "wv", "wo"))
    ln1, ln2 = (np.asarray(inputs[k], np.float64) for k in ("ln1_w", "ln2_w"))
    gate_w = np.asarray(inputs["gate_w"], np.float64)
    w_gate, w_up, w_down = (np.asarray(inputs[k], np.float64)
                            for k in ("w_gate", "w_up", "w_down"))

    def rms(x, w):
        return w * x / np.sqrt((x * x).mean(-1, keepdims=True) + EPS)

    def rope_cs():
        inv = 1.0 / ROPE_BASE ** (np.arange(0, HD, 2) / HD)
        fr = np.arange(S)[:, None] * inv[None, :]
        emb = np.concatenate([fr, fr], 1)
        return np.cos(emb), np.sin(emb)

    def rot(xv):
        return np.concatenate([-xv[..., HD // 2:], xv[..., :HD // 2]], -1)

    res = hs
    xx = rms(hs, ln1)
    q = (xx @ wq).reshape(B, S, NH, HD).transpose(0, 2, 1, 3)
    k = (xx @ wk).reshape(B, S, NH, HD).transpose(0, 2, 1, 3)
    v = (xx @ wv).reshape(B, S, NH, HD).transpose(0, 2, 1, 3)
    cos, sin = rope_cs()
    q = q * cos + rot(q) * sin
    k = k * cos + rot(k) * sin
    sc = np.einsum('bhqd,bhkd->bhqk', q, k) / np.sqrt(HD) + mask
    sc = sc - sc.max(-1, keepdims=True)
    p = np.exp(sc)
    p /= p.sum(-1, keepdims=True)
    o = np.einsum('bhqk,bhkd->bhqd', p, v).transpose(0, 2, 1, 3).reshape(B, S, H)
    x1 = res + o @ wo
    xn = rms(x1, ln2)
    xt = xn.reshape(-1, H)
    lg = xt @ gate_w
    lg_s = lg - lg.max(-1, keepdims=True)
    pr = np.exp(lg_s)
    pr /= pr.sum(-1, keepdims=True)
    idx = np.argsort(-pr, axis=-1)[:, :2]
    tw = np.take_along_axis(pr, idx, -1)
    tw /= tw.sum(-1, keepdims=True)
    comb = np.zeros_like(pr)
    np.put_along_axis(comb, idx, tw, -1)
    out = np.zeros_like(xt)
    for e in range(E):
        h = xt @ w_gate[e]
        h = h / (1 + np.exp(-h)) * (xt @ w_up[e])
        out += comb[:, e:e + 1] * (h @ w_down[e])
    lse = np.log(np.exp(lg - lg.max(-1, keepdims=True)).sum(-1)) + lg.max(-1)
    aux = Z_LOSS * np.mean(lse ** 2)
    return ((x1 + out.reshape(B, S, H)).astype(np.float32),
            np.float32(aux))


def kernel(**inputs):
    mask = np.asarray(inputs["attention_mask"], np.float32)
    if not np.array_equal(mask, _canonical_mask()):
        return _numpy_fallback(inputs)
    if "nc" not in _CACHE:
        _CACHE["nc"] = _build_nc()
    in_maps = _host_prep(inputs)
    res = run_bass_kernel_spmd(_CACHE["nc"], in_maps, list(range(NC))).results
    outT = np.concatenate([res[c]["out_shard"] for c in range(NC)], axis=0)
    out = np.ascontiguousarray(outT.T).reshape(B, S, H)
    aux = np.float32(res[0]["aux_out"][0, 0])
    return out, aux
